# revision 11
# baseline (speedup 1.0000x reference)
import numpy as np
import ml_dtypes

B = 8
NC = 8
IMG = 1024
N0 = 1024
D0 = 1024
BN_EPS = 1e-3
ALPHA = 0.3

_CACHE = {}



def _perms():
    i = np.arange(1024)
    u, v, a, bh = i // 512, (i // 256) % 2, (i // 16) % 16, i % 16
    rho = 512 * u + 32 * a + 16 * v + bh
    p = np.arange(512)
    ocb, p1 = p // 128, p % 128
    j0, uu, vv = p1 % 32, p1 // 64, (p1 // 32) % 2
    sigma = 128 * ocb + 64 * uu + 2 * j0 + vv
    return rho, sigma


def host_prep(inputs):
    bf16 = ml_dtypes.bfloat16
    X = np.asarray(inputs["X"], np.float32).reshape(B, IMG, IMG)
    W_emb = np.asarray(inputs["W_emb"], np.float32)
    conv_w = np.asarray(inputs["conv_w"], np.float32)
    conv_b = np.asarray(inputs["conv_b"], np.float32)
    g = np.asarray(inputs["bn_gamma"], np.float32)
    be = np.asarray(inputs["bn_beta"], np.float32)
    mu = np.asarray(inputs["bn_mean"], np.float32)
    var = np.asarray(inputs["bn_var"], np.float32)
    W_dense = np.asarray(inputs["W_dense"], np.float32)
    b_dense = np.asarray(inputs["b_dense"], np.float32)

    rho, sigma = _perms()

    s_all = g / np.sqrt(var + BN_EPS)
    t_all = (conv_b - mu) * s_all + be

    peT = np.zeros((N0, 33, 33), np.float32)
    peT[:, :32, :32] = W_emb.reshape(N0, 32, 32)
    pe_host = np.ascontiguousarray(
        peT.reshape(8, 128, 1089)).astype(bf16)

    wd_host = np.ascontiguousarray(W_dense[rho, :].reshape(8, 128, D0))

    ident = np.eye(128, dtype=np.float32)
    bd_host = b_dense.reshape(1, D0).astype(np.float32)

    ocb_pos = np.arange(512) // 128
    p1_pos = np.arange(512) % 128

    in_maps = []
    for k in range(NC):
        ch = 512 * k + sigma
        cw = conv_w[:, :, :, ch].reshape(72, 128, 512).astype(bf16)
        st = np.zeros((128, 8), np.float32)
        st[p1_pos, 2 * ocb_pos] = s_all[ch]
        st[p1_pos, 2 * ocb_pos + 1] = t_all[ch]
        Xs = X[:, 128 * k:128 * (k + 1), :]
        axt = Xs.reshape(B, 4, 32, 32, 32).transpose(0, 2, 4, 1, 3).reshape(B, 1024, 128)
        axt = axt[:, rho, :].reshape(B, 8, 128, 128).transpose(0, 2, 1, 3)
        axt = np.ascontiguousarray(axt.reshape(B, 128, 1024))
        wrows = np.ascontiguousarray(W_emb[128 * k:128 * (k + 1), :][:, rho])
        in_maps.append({
            "cw": cw, "pe": pe_host, "st": st, "axt": axt,
            "wrows": wrows, "wd": wd_host, "ident": ident, "bd": bd_host,
            "ones": np.ones((1, 128), np.float32),
        })
    return in_maps



def _build():
    import concourse.bass as bass
    import concourse.tile as tile
    import concourse.mybir as mybir
    from concourse import bacc

    f32 = mybir.dt.float32
    f32r = mybir.dt.float32r
    bf16 = mybir.dt.bfloat16
    Alu = mybir.AluOpType
    Act = mybir.ActivationFunctionType

    nc = bacc.Bacc("TRN2", target_bir_lowering=False, debug=False)

    cw_d = nc.dram_tensor("cw", [72, 128, 512], bf16, kind="ExternalInput").ap()
    pe_d = nc.dram_tensor("pe", [8, 128, 1089], bf16, kind="ExternalInput").ap()
    st_d = nc.dram_tensor("st", [128, 8], f32, kind="ExternalInput").ap()
    axt_d = nc.dram_tensor("axt", [8, 128, 1024], f32r, kind="ExternalInput").ap()
    wrows_d = nc.dram_tensor("wrows", [128, 1024], f32, kind="ExternalInput").ap()
    wd_d = nc.dram_tensor("wd", [8, 128, 1024], f32r, kind="ExternalInput").ap()
    ident_d = nc.dram_tensor("ident", [128, 128], f32, kind="ExternalInput").ap()
    bd_d = nc.dram_tensor("bd", [1, 1024], f32r, kind="ExternalInput").ap()
    ones_d = nc.dram_tensor("ones", [1, 128], f32r, kind="ExternalInput").ap()
    out_d = nc.dram_tensor("out", [8, 128, 1024], f32, kind="ExternalOutput").ap()

    with tile.TileContext(nc) as tc:
        with (
            tc.tile_pool(name="kpool", bufs=1) as kpool,
            tc.tile_pool(name="cwpool", bufs=8) as cwpool,
            tc.tile_pool(name="zpool", bufs=2) as zpool,
            tc.tile_pool(name="opool", bufs=3) as opool,
        ):
            pe_sb = kpool.tile([128, 8 * 1089], bf16, tag="pe")
            for kt in range(8):
                nc.sync.dma_start(pe_sb[:, kt * 1089:(kt + 1) * 1089], pe_d[kt])
            st_sb = kpool.tile([128, 8], f32, tag="st")
            nc.sync.dma_start(st_sb[:], st_d[:])

            Y = kpool.tile([128, 1024], f32, tag="Y")
            crows = kpool.tile([128, 1024], f32, tag="crows")
            crows2 = kpool.tile([128, 1024], f32, tag="crows2")
            lhsC = kpool.tile([128, 1024], f32r, tag="lhsC")
            drows = kpool.tile([128, 1024], f32, tag="drows")

            wrows_sb = kpool.tile([128, 1024], f32, tag="wrows")
            nc.sync.dma_start(wrows_sb[:], wrows_d[:])
            ident_sb = kpool.tile([128, 128], f32, tag="ident")
            nc.sync.dma_start(ident_sb[:], ident_d[:])
            bd_sb = kpool.tile([1, 1024], f32r, tag="bd")
            nc.sync.dma_start(bd_sb[:], bd_d[:])
            ones_sb = kpool.tile([1, 128], f32r, tag="ones")
            nc.sync.dma_start(ones_sb[:], ones_d[:])

            wd_sb = kpool.tile([128, 8 * 1024], f32r, tag="wd")
            for kt in range(8):
                nc.sync.dma_start(wd_sb[:, kt * 1024:(kt + 1) * 1024], wd_d[kt])
            axt_sb = kpool.tile([128, 8 * 1024], f32r, tag="axt")
            for b in range(8):
                nc.sync.dma_start(axt_sb[:, b * 1024:(b + 1) * 1024], axt_d[b])

            pe3 = pe_sb.rearrange("p (k a b) -> p k a b", k=8, a=33, b=33)
            with tc.tile_pool(name="psA", bufs=1, space="PSUM") as psA, \
                 tc.tile_pool(name="psT", bufs=2, space="PSUM") as psT:
                yps = [psA.tile([128, 256], f32, tag=f"y{ocb}", name=f"yps{ocb}")
                       for ocb in range(4)]
                idx = 0
                for dd in range(9):
                    di, dj = dd // 3, dd % 3
                    for kt in range(8):
                        cw_t = cwpool.tile([128, 512], bf16, tag="cw")
                        nc.sync.dma_start(cw_t[:], cw_d[dd * 8 + kt])
                        rhs = pe3[:, kt, di:di + 31:2, dj:dj + 31:2]
                        for ocb in range(4):
                            nc.tensor.matmul(
                                yps[ocb][:],
                                cw_t[:, ocb * 128:(ocb + 1) * 128],
                                rhs,
                                start=(idx == 0), stop=(idx == 71))
                        idx += 1

                for ocb in range(4):
                    z = zpool.tile([128, 256], f32, tag="z")
                    nc.scalar.activation(
                        z[:], yps[ocb][:], Act.Identity,
                        bias=st_sb[:, 2 * ocb + 1:2 * ocb + 2],
                        scale=st_sb[:, 2 * ocb:2 * ocb + 1])
                    nc.vector.scalar_tensor_tensor(
                        Y[:, 256 * ocb:256 * (ocb + 1)],
                        z[:], ALPHA, z[:], Alu.mult, Alu.max)

                Y4 = Y.rearrange("p (o a b) -> p o a b", o=4, a=16, b=16)
                cr4 = crows.rearrange("p (u v c) -> p u v c", u=2, v=2, c=256)
                for ocb in range(4):
                    for u in range(2):
                        for v in range(2):
                            pbase = 32 * (2 * u + v)
                            src = Y4[pbase:pbase + 32, ocb]
                            dst = cr4[32 * ocb:32 * (ocb + 1), u, v]
                            nc.sync.dma_start(dst, src)

                nc.vector.tensor_tensor(crows2[:], crows[:], wrows_sb[:], Alu.add)

                for k2 in range(8):
                    tp = psT.tile([128, 128], f32, tag="tp")
                    nc.tensor.transpose(
                        tp[:], crows2[:, 128 * k2:128 * (k2 + 1)], ident_sb[:])
                    nc.vector.tensor_copy(lhsC[:, 128 * k2:128 * (k2 + 1)], tp[:])

            with tc.tile_pool(name="psD", bufs=1, space="PSUM") as psD, \
                 tc.tile_pool(name="psF", bufs=3, space="PSUM") as psF:
                dps = psD.tile([128, 1024], f32, tag="dps")
                for nb in range(2):
                    o = dps[:, 512 * nb:512 * (nb + 1)]
                    for kt in range(8):
                        nc.tensor.matmul(
                            o,
                            lhsC[:, 128 * kt:128 * (kt + 1)],
                            wd_sb[:, 1024 * kt + 512 * nb:1024 * kt + 512 * nb + 512],
                            start=(kt == 0), stop=False)
                    nc.tensor.matmul(
                        o, ones_sb[:],
                        bd_sb[:, 512 * nb:512 * (nb + 1)],
                        start=False, stop=True)
                nc.vector.tensor_copy(drows[:], dps[:])

                for b in range(8):
                    fp = psF.tile([128, 1024], f32, tag="fp")
                    for nb in range(2):
                        o = fp[:, 512 * nb:512 * (nb + 1)]
                        for kt in range(8):
                            nc.tensor.matmul(
                                o,
                                axt_sb[:, 1024 * b + 128 * kt:1024 * b + 128 * (kt + 1)],
                                wd_sb[:, 1024 * kt + 512 * nb:1024 * kt + 512 * nb + 512],
                                start=(kt == 0), stop=(kt == 7))
                    ot = opool.tile([128, 1024], f32, tag="ot")
                    nc.vector.tensor_tensor(ot[:], fp[:], drows[:], Alu.add)
                    nc.sync.dma_start(out_d[b], ot[:])

    nc.compile()
    return nc


def get_nc():
    if "nc" not in _CACHE:
        _CACHE["nc"] = _build()
    return _CACHE["nc"]



def run(inputs, trace=False, **kwargs):
    from concourse.bass_utils import run_bass_kernel_spmd
    nc = get_nc()
    in_maps = host_prep(inputs)
    res = run_bass_kernel_spmd(nc, in_maps, list(range(NC)), trace=trace, **kwargs)
    out = np.empty((B, N0, D0), np.float32)
    for k in range(NC):
        out[:, 128 * k:128 * (k + 1), :] = res.results[k]["out"]
    return out, res


def kernel(**inputs):
    out, _ = run(inputs)
    return out


# revision 12
# speedup vs baseline: 1.2688x; 1.2688x over previous
import numpy as np
import ml_dtypes

B = 8
NC = 8
IMG = 1024
N0 = 1024
D0 = 1024
BN_EPS = 1e-3
ALPHA = 0.3

_CACHE = {}



def _perms():
    i = np.arange(1024)
    u, v, a, bh = i // 512, (i // 256) % 2, (i // 16) % 16, i % 16
    rho = 512 * u + 32 * a + 16 * v + bh
    p = np.arange(512)
    ocb, p1 = p // 128, p % 128
    j0, uu, vv = p1 % 32, p1 // 64, (p1 // 32) % 2
    sigma = 128 * ocb + 64 * uu + 2 * j0 + vv
    return rho, sigma


def host_prep(inputs):
    bf16 = ml_dtypes.bfloat16
    X = np.asarray(inputs["X"], np.float32).reshape(B, IMG, IMG)
    W_emb = np.asarray(inputs["W_emb"], np.float32)
    conv_w = np.asarray(inputs["conv_w"], np.float32)
    conv_b = np.asarray(inputs["conv_b"], np.float32)
    g = np.asarray(inputs["bn_gamma"], np.float32)
    be = np.asarray(inputs["bn_beta"], np.float32)
    mu = np.asarray(inputs["bn_mean"], np.float32)
    var = np.asarray(inputs["bn_var"], np.float32)
    W_dense = np.asarray(inputs["W_dense"], np.float32)
    b_dense = np.asarray(inputs["b_dense"], np.float32)

    rho, sigma = _perms()

    s_all = g / np.sqrt(var + BN_EPS)
    t_all = (conv_b - mu) * s_all + be

    peT = np.zeros((N0, 33, 33), np.float32)
    peT[:, :32, :32] = W_emb.reshape(N0, 32, 32)
    pe_host = np.ascontiguousarray(
        peT.reshape(8, 128, 1089)).astype(bf16)

    wd_host = np.ascontiguousarray(W_dense[rho, :].reshape(8, 128, D0))

    ident = np.eye(128, dtype=np.float32)
    bd_host = b_dense.reshape(1, D0).astype(np.float32)

    ocb_pos = np.arange(512) // 128
    p1_pos = np.arange(512) % 128

    in_maps = []
    for k in range(NC):
        ch = 512 * k + sigma
        cw = conv_w[:, :, :, ch].reshape(18, 4, 128, 512).transpose(0, 2, 1, 3)
        cw = np.ascontiguousarray(cw.reshape(18, 128, 2048)).astype(bf16)
        st = np.zeros((128, 8), np.float32)
        st[p1_pos, 2 * ocb_pos] = s_all[ch]
        st[p1_pos, 2 * ocb_pos + 1] = t_all[ch]
        Xs = X[:, 128 * k:128 * (k + 1), :]
        axt = Xs.reshape(B, 4, 32, 32, 32).transpose(0, 2, 4, 1, 3).reshape(B, 1024, 128)
        axt = axt[:, rho, :].reshape(B, 8, 128, 128).transpose(0, 2, 1, 3)
        axt = np.ascontiguousarray(axt.reshape(B, 128, 1024))
        wrows = np.ascontiguousarray(W_emb[128 * k:128 * (k + 1), :][:, rho])
        in_maps.append({
            "cw": cw, "pe": pe_host, "st": st, "axt": axt,
            "wrows": wrows, "wd": wd_host, "ident": ident, "bd": bd_host,
            "ones": np.ones((1, 128), np.float32),
        })
    return in_maps



def _build():
    import concourse.bass as bass
    import concourse.tile as tile
    import concourse.mybir as mybir
    from concourse import bacc

    f32 = mybir.dt.float32
    f32r = mybir.dt.float32r
    bf16 = mybir.dt.bfloat16
    Alu = mybir.AluOpType
    Act = mybir.ActivationFunctionType

    nc = bacc.Bacc("TRN2", target_bir_lowering=False, debug=False)

    cw_d = nc.dram_tensor("cw", [18, 128, 2048], bf16, kind="ExternalInput").ap()
    pe_d = nc.dram_tensor("pe", [8, 128, 1089], bf16, kind="ExternalInput").ap()
    st_d = nc.dram_tensor("st", [128, 8], f32, kind="ExternalInput").ap()
    axt_d = nc.dram_tensor("axt", [8, 128, 1024], f32r, kind="ExternalInput").ap()
    wrows_d = nc.dram_tensor("wrows", [128, 1024], f32, kind="ExternalInput").ap()
    wd_d = nc.dram_tensor("wd", [8, 128, 1024], f32r, kind="ExternalInput").ap()
    ident_d = nc.dram_tensor("ident", [128, 128], f32, kind="ExternalInput").ap()
    bd_d = nc.dram_tensor("bd", [1, 1024], f32r, kind="ExternalInput").ap()
    ones_d = nc.dram_tensor("ones", [1, 128], f32r, kind="ExternalInput").ap()
    out_d = nc.dram_tensor("out", [8, 128, 1024], f32, kind="ExternalOutput").ap()

    with tile.TileContext(nc) as tc:
        with (
            tc.tile_pool(name="kpool", bufs=1) as kpool,
            tc.tile_pool(name="cwpool", bufs=6) as cwpool,
            tc.tile_pool(name="zpool", bufs=2) as zpool,
            tc.tile_pool(name="opool", bufs=3) as opool,
        ):
            pe_sb = kpool.tile([128, 8 * 1089], bf16, tag="pe")
            st_sb = kpool.tile([128, 8], f32, tag="st")
            Y = kpool.tile([128, 1024], f32, tag="Y")
            crows = kpool.tile([128, 1024], f32, tag="crows")
            crows2 = kpool.tile([128, 1024], f32, tag="crows2")
            lhsC = kpool.tile([128, 1024], f32r, tag="lhsC")
            drows = kpool.tile([128, 1024], f32, tag="drows")
            wrows_sb = kpool.tile([128, 1024], f32, tag="wrows")
            ident_sb = kpool.tile([128, 128], f32, tag="ident")
            bd_sb = kpool.tile([1, 1024], f32r, tag="bd")
            ones_sb = kpool.tile([1, 128], f32r, tag="ones")
            wd_sb = kpool.tile([128, 8 * 1024], f32r, tag="wd")
            axt_sb = kpool.tile([128, 8 * 1024], f32r, tag="axt")

            pe3 = pe_sb.rearrange("p (k a b) -> p k a b", k=8, a=33, b=33)
            with tc.tile_pool(name="psA", bufs=1, space="PSUM") as psA:
                yps = [psA.tile([128, 256], f32, tag=f"y{ocb}", name=f"yps{ocb}")
                       for ocb in range(4)]
                for g in range(18):
                    dd = g // 2
                    di, dj = dd // 3, dd % 3
                    cw_t = cwpool.tile([128, 2048], bf16, tag="cw")
                    nc.sync.dma_start(cw_t[:], cw_d[g])
                    if g == 0:
                        for kt in range(8):
                            nc.sync.dma_start(
                                pe_sb[:, kt * 1089:(kt + 1) * 1089], pe_d[kt])
                        nc.sync.dma_start(st_sb[:], st_d[:])
                        nc.sync.dma_start(wrows_sb[:], wrows_d[:])
                        nc.sync.dma_start(ident_sb[:], ident_d[:])
                        nc.sync.dma_start(bd_sb[:], bd_d[:])
                        nc.sync.dma_start(ones_sb[:], ones_d[:])
                    elif g == 1:
                        nc.sync.dma_start(axt_sb[:, 0:1024], axt_d[0])
                        nc.sync.dma_start(axt_sb[:, 1024:2048], axt_d[1])
                    elif 2 <= g <= 9:
                        kt = g - 2
                        nc.sync.dma_start(
                            wd_sb[:, kt * 1024:(kt + 1) * 1024], wd_d[kt])
                    elif 10 <= g <= 15:
                        b = g - 8
                        nc.sync.dma_start(
                            axt_sb[:, b * 1024:(b + 1) * 1024], axt_d[b])
                    for tt in range(4):
                        kt = 4 * (g % 2) + tt
                        rhs = pe3[:, kt, di:di + 31:2, dj:dj + 31:2]
                        for ocb in range(4):
                            nc.tensor.matmul(
                                yps[ocb][:],
                                cw_t[:, 512 * tt + 128 * ocb:512 * tt + 128 * (ocb + 1)],
                                rhs,
                                start=(g == 0 and tt == 0),
                                stop=(g == 17 and tt == 3))

                for ocb in range(4):
                    z = zpool.tile([128, 256], f32, tag="z")
                    nc.scalar.activation(
                        z[:], yps[ocb][:], Act.Identity,
                        bias=st_sb[:, 2 * ocb + 1:2 * ocb + 2],
                        scale=st_sb[:, 2 * ocb:2 * ocb + 1])
                    nc.vector.scalar_tensor_tensor(
                        Y[:, 256 * ocb:256 * (ocb + 1)],
                        z[:], ALPHA, z[:], Alu.mult, Alu.max)

            Y4 = Y.rearrange("p (o a b) -> p o a b", o=4, a=16, b=16)
            cr4 = crows.rearrange("p (u v c) -> p u v c", u=2, v=2, c=256)
            for ocb in range(4):
                for u in range(2):
                    for v in range(2):
                        pbase = 32 * (2 * u + v)
                        nc.scalar.dma_start(
                            cr4[32 * ocb:32 * (ocb + 1), u, v],
                            Y4[pbase:pbase + 32, ocb])

            nc.vector.tensor_tensor(crows2[:], crows[:], wrows_sb[:], Alu.add)

            with tc.tile_pool(name="psT", bufs=2, space="PSUM") as psT:
                for k2 in range(8):
                    tp = psT.tile([128, 128], f32, tag="tp")
                    nc.tensor.transpose(
                        tp[:], crows2[:, 128 * k2:128 * (k2 + 1)], ident_sb[:])
                    nc.vector.tensor_copy(lhsC[:, 128 * k2:128 * (k2 + 1)], tp[:])

            with tc.tile_pool(name="psD", bufs=1, space="PSUM") as psD, \
                 tc.tile_pool(name="psF", bufs=3, space="PSUM") as psF:
                dps = psD.tile([128, 1024], f32, tag="dps")
                fp0 = psF.tile([128, 1024], f32, tag="fp", name="fp0")
                fp1 = psF.tile([128, 1024], f32, tag="fp", name="fp1")

                def wslice(kt, nb):
                    o = 1024 * kt + 512 * nb
                    return wd_sb[:, o:o + 512]

                def aslice(b, kt):
                    o = 1024 * b + 128 * kt
                    return axt_sb[:, o:o + 128]

                for kt in range(8):
                    for nb in range(2):
                        nc.tensor.matmul(
                            dps[:, 512 * nb:512 * (nb + 1)],
                            lhsC[:, 128 * kt:128 * (kt + 1)], wslice(kt, nb),
                            start=(kt == 0), stop=False)
                        nc.tensor.matmul(
                            fp0[:, 512 * nb:512 * (nb + 1)],
                            aslice(0, kt), wslice(kt, nb),
                            start=(kt == 0), stop=(kt == 7))
                        nc.tensor.matmul(
                            fp1[:, 512 * nb:512 * (nb + 1)],
                            aslice(1, kt), wslice(kt, nb),
                            start=(kt == 0), stop=(kt == 7))
                for nb in range(2):
                    nc.tensor.matmul(
                        dps[:, 512 * nb:512 * (nb + 1)], ones_sb[:],
                        bd_sb[:, 512 * nb:512 * (nb + 1)],
                        start=False, stop=True)
                nc.vector.tensor_copy(drows[:], dps[:])

                for b in range(2):
                    fp = fp0 if b == 0 else fp1
                    ot = opool.tile([128, 1024], f32, tag="ot", name=f"ot{b}")
                    nc.vector.tensor_tensor(ot[:], fp[:], drows[:], Alu.add)
                    nc.scalar.dma_start(out_d[b], ot[:])

                for b in range(2, 8):
                    fp = psF.tile([128, 1024], f32, tag="fp", name=f"fp{b}")
                    for nb in range(2):
                        for kt in range(8):
                            nc.tensor.matmul(
                                fp[:, 512 * nb:512 * (nb + 1)],
                                aslice(b, kt), wslice(kt, nb),
                                start=(kt == 0), stop=(kt == 7))
                    ot = opool.tile([128, 1024], f32, tag="ot", name=f"ot{b}")
                    nc.vector.tensor_tensor(ot[:], fp[:], drows[:], Alu.add)
                    nc.scalar.dma_start(out_d[b], ot[:])

    nc.compile()
    return nc


def get_nc():
    if "nc" not in _CACHE:
        _CACHE["nc"] = _build()
    return _CACHE["nc"]



def run(inputs, trace=False, **kwargs):
    from concourse.bass_utils import run_bass_kernel_spmd
    nc = get_nc()
    in_maps = host_prep(inputs)
    res = run_bass_kernel_spmd(nc, in_maps, list(range(NC)), trace=trace, **kwargs)
    out = np.empty((B, N0, D0), np.float32)
    for k in range(NC):
        out[:, 128 * k:128 * (k + 1), :] = res.results[k]["out"]
    return out, res


def kernel(**inputs):
    out, _ = run(inputs)
    return out


# revision 13
# speedup vs baseline: 1.3738x; 1.0828x over previous
import numpy as np
import ml_dtypes

B = 8
NC = 8
IMG = 1024
N0 = 1024
D0 = 1024
BN_EPS = 1e-3
ALPHA = 0.3

_CACHE = {}



def _perms():
    i = np.arange(1024)
    u, v, a, bh = i // 512, (i // 256) % 2, (i // 16) % 16, i % 16
    rho = 512 * u + 32 * a + 16 * v + bh
    p = np.arange(512)
    ocb, p1 = p // 128, p % 128
    j0, uu, vv = p1 % 32, p1 // 64, (p1 // 32) % 2
    sigma = 128 * ocb + 64 * uu + 2 * j0 + vv
    return rho, sigma


def host_prep(inputs):
    bf16 = ml_dtypes.bfloat16
    X = np.asarray(inputs["X"], np.float32).reshape(B, IMG, IMG)
    W_emb = np.asarray(inputs["W_emb"], np.float32)
    conv_w = np.asarray(inputs["conv_w"], np.float32)
    conv_b = np.asarray(inputs["conv_b"], np.float32)
    g = np.asarray(inputs["bn_gamma"], np.float32)
    be = np.asarray(inputs["bn_beta"], np.float32)
    mu = np.asarray(inputs["bn_mean"], np.float32)
    var = np.asarray(inputs["bn_var"], np.float32)
    W_dense = np.asarray(inputs["W_dense"], np.float32)
    b_dense = np.asarray(inputs["b_dense"], np.float32)

    rho, sigma = _perms()

    s_all = g / np.sqrt(var + BN_EPS)
    t_all = (conv_b - mu) * s_all + be

    peT = np.zeros((N0, 33, 33), np.float32)
    peT[:, :32, :32] = W_emb.reshape(N0, 32, 32)
    pe_host = np.ascontiguousarray(
        peT.reshape(8, 128, 1089)).astype(bf16)

    wd_host = np.ascontiguousarray(W_dense[rho, :].reshape(8, 128, D0))

    ident = np.eye(128, dtype=np.float32)
    bd_host = b_dense.reshape(1, D0).astype(np.float32)

    ocb_pos = np.arange(512) // 128
    p1_pos = np.arange(512) % 128

    in_maps = []
    for k in range(NC):
        ch = 512 * k + sigma
        cw = conv_w[:, :, :, ch].reshape(18, 4, 128, 512).transpose(0, 2, 1, 3)
        cw = np.ascontiguousarray(cw.reshape(18, 128, 2048)).astype(bf16)
        st = np.zeros((128, 8), np.float32)
        st[p1_pos, 2 * ocb_pos] = s_all[ch]
        st[p1_pos, 2 * ocb_pos + 1] = t_all[ch]
        Xs = X[:, 128 * k:128 * (k + 1), :]
        axt = Xs.reshape(B, 4, 32, 32, 32).transpose(0, 2, 4, 1, 3).reshape(B, 1024, 128)
        axt = axt[:, rho, :].reshape(B, 8, 128, 128).transpose(0, 2, 1, 3)
        axt = np.ascontiguousarray(axt.reshape(B, 128, 1024))
        wr = W_emb[128 * k:128 * (k + 1), :][:, rho]
        wrows = np.ascontiguousarray(
            wr.T.reshape(8, 128, 128).transpose(1, 0, 2).reshape(128, 1024))
        in_maps.append({
            "cw": cw, "pe": pe_host, "st": st, "axt": axt,
            "wrows": wrows, "wd": wd_host, "ident": ident, "bd": bd_host,
            "ones": np.ones((1, 128), np.float32),
        })
    return in_maps



def _build():
    import concourse.bass as bass
    import concourse.tile as tile
    import concourse.mybir as mybir
    from concourse import bacc

    f32 = mybir.dt.float32
    f32r = mybir.dt.float32r
    bf16 = mybir.dt.bfloat16
    Alu = mybir.AluOpType
    Act = mybir.ActivationFunctionType

    nc = bacc.Bacc("TRN2", target_bir_lowering=False, debug=False)

    cw_d = nc.dram_tensor("cw", [18, 128, 2048], bf16, kind="ExternalInput").ap()
    pe_d = nc.dram_tensor("pe", [8, 128, 1089], bf16, kind="ExternalInput").ap()
    st_d = nc.dram_tensor("st", [128, 8], f32, kind="ExternalInput").ap()
    axt_d = nc.dram_tensor("axt", [8, 128, 1024], f32r, kind="ExternalInput").ap()
    wrows_d = nc.dram_tensor("wrows", [128, 1024], f32, kind="ExternalInput").ap()
    wd_d = nc.dram_tensor("wd", [8, 128, 1024], f32r, kind="ExternalInput").ap()
    ident_d = nc.dram_tensor("ident", [128, 128], f32, kind="ExternalInput").ap()
    bd_d = nc.dram_tensor("bd", [1, 1024], f32r, kind="ExternalInput").ap()
    ones_d = nc.dram_tensor("ones", [1, 128], f32r, kind="ExternalInput").ap()
    out_d = nc.dram_tensor("out", [8, 128, 1024], f32, kind="ExternalOutput").ap()

    with tile.TileContext(nc) as tc:
        with (
            tc.tile_pool(name="kpool", bufs=1) as kpool,
            tc.tile_pool(name="cwpool", bufs=6) as cwpool,
            tc.tile_pool(name="zpool", bufs=2) as zpool,
            tc.tile_pool(name="opool", bufs=3) as opool,
        ):
            pe_sb = kpool.tile([128, 8 * 1089], bf16, tag="pe")
            st_sb = kpool.tile([128, 8], f32, tag="st")
            Y = kpool.tile([128, 1024], f32, tag="Y")
            crows = kpool.tile([128, 1024], f32, tag="crows")
            lhsC = kpool.tile([128, 1024], f32r, tag="lhsC")
            drows = kpool.tile([128, 1024], f32, tag="drows")
            wrows_sb = kpool.tile([128, 1024], f32, tag="wrows")
            ident_sb = kpool.tile([128, 128], f32, tag="ident")
            bd_sb = kpool.tile([1, 1024], f32r, tag="bd")
            ones_sb = kpool.tile([1, 128], f32r, tag="ones")
            wd_sb = kpool.tile([128, 8 * 1024], f32r, tag="wd")
            axt_sb = kpool.tile([128, 8 * 1024], f32r, tag="axt")

            pe3 = pe_sb.rearrange("p (k a b) -> p k a b", k=8, a=33, b=33)
            with tc.tile_pool(name="psA", bufs=1, space="PSUM") as psA:
                yps = [psA.tile([128, 256], f32, tag=f"y{ocb}", name=f"yps{ocb}")
                       for ocb in range(4)]
                for g in range(18):
                    dd = g // 2
                    di, dj = dd // 3, dd % 3
                    cw_t = cwpool.tile([128, 2048], bf16, tag="cw")
                    nc.sync.dma_start(cw_t[:], cw_d[g])
                    if g == 0:
                        for kt in range(4):
                            nc.sync.dma_start(
                                pe_sb[:, kt * 1089:(kt + 1) * 1089], pe_d[kt])
                        nc.sync.dma_start(st_sb[:], st_d[:])
                    elif g == 1:
                        for kt in range(4, 8):
                            nc.sync.dma_start(
                                pe_sb[:, kt * 1089:(kt + 1) * 1089], pe_d[kt])
                    elif g == 3:
                        nc.sync.dma_start(axt_sb[:, 0:1024], axt_d[0])
                    elif g == 5:
                        nc.sync.dma_start(axt_sb[:, 1024:2048], axt_d[1])
                    elif g == 7:
                        nc.sync.dma_start(wrows_sb[:], wrows_d[:])
                    elif g == 9:
                        nc.sync.dma_start(ident_sb[:], ident_d[:])
                        nc.sync.dma_start(bd_sb[:], bd_d[:])
                        nc.sync.dma_start(ones_sb[:], ones_d[:])
                    elif g == 11:
                        nc.sync.dma_start(wd_sb[:, 0:1024], wd_d[0])
                    elif g == 13:
                        nc.sync.dma_start(wd_sb[:, 1024:2048], wd_d[1])
                    for tt in range(4):
                        kt = 4 * (g % 2) + tt
                        rhs = pe3[:, kt, di:di + 31:2, dj:dj + 31:2]
                        for ocb in range(4):
                            nc.tensor.matmul(
                                yps[ocb][:],
                                cw_t[:, 512 * tt + 128 * ocb:512 * tt + 128 * (ocb + 1)],
                                rhs,
                                start=(g == 0 and tt == 0),
                                stop=(g == 17 and tt == 3))

                for kt in range(2, 8):
                    nc.sync.dma_start(
                        wd_sb[:, kt * 1024:(kt + 1) * 1024], wd_d[kt])
                for b in range(2, 8):
                    nc.sync.dma_start(
                        axt_sb[:, b * 1024:(b + 1) * 1024], axt_d[b])

                for ocb in range(4):
                    z = zpool.tile([128, 256], f32, tag="z")
                    nc.scalar.activation(
                        z[:], yps[ocb][:], Act.Identity,
                        bias=st_sb[:, 2 * ocb + 1:2 * ocb + 2],
                        scale=st_sb[:, 2 * ocb:2 * ocb + 1])
                    nc.vector.scalar_tensor_tensor(
                        Y[:, 256 * ocb:256 * (ocb + 1)],
                        z[:], ALPHA, z[:], Alu.mult, Alu.max)

            Y4 = Y.rearrange("p (o a b) -> p o a b", o=4, a=16, b=16)
            cr4 = crows.rearrange("p (u v c) -> p u v c", u=2, v=2, c=256)
            for ocb in range(4):
                for u in range(2):
                    for v in range(2):
                        pbase = 32 * (2 * u + v)
                        nc.scalar.dma_start(
                            cr4[32 * ocb:32 * (ocb + 1), u, v],
                            Y4[pbase:pbase + 32, ocb])

            with tc.tile_pool(name="psT", bufs=2, space="PSUM") as psT:
                for k2 in range(8):
                    tp = psT.tile([128, 128], f32, tag="tp")
                    nc.tensor.transpose(
                        tp[:], crows[:, 128 * k2:128 * (k2 + 1)], ident_sb[:])
                    nc.vector.tensor_tensor(
                        lhsC[:, 128 * k2:128 * (k2 + 1)], tp[:],
                        wrows_sb[:, 128 * k2:128 * (k2 + 1)], Alu.add)

            with tc.tile_pool(name="psD", bufs=1, space="PSUM") as psD, \
                 tc.tile_pool(name="psF", bufs=3, space="PSUM") as psF:
                dps = psD.tile([128, 1024], f32, tag="dps")
                fp0 = psF.tile([128, 1024], f32, tag="fp", name="fp0")
                fp1 = psF.tile([128, 1024], f32, tag="fp", name="fp1")

                def wslice(kt, nb):
                    o = 1024 * kt + 512 * nb
                    return wd_sb[:, o:o + 512]

                def aslice(b, kt):
                    o = 1024 * b + 128 * kt
                    return axt_sb[:, o:o + 128]

                for kt in range(8):
                    for nb in range(2):
                        nc.tensor.matmul(
                            dps[:, 512 * nb:512 * (nb + 1)],
                            lhsC[:, 128 * kt:128 * (kt + 1)], wslice(kt, nb),
                            start=(kt == 0), stop=False)
                        nc.tensor.matmul(
                            fp0[:, 512 * nb:512 * (nb + 1)],
                            aslice(0, kt), wslice(kt, nb),
                            start=(kt == 0), stop=(kt == 7))
                        nc.tensor.matmul(
                            fp1[:, 512 * nb:512 * (nb + 1)],
                            aslice(1, kt), wslice(kt, nb),
                            start=(kt == 0), stop=(kt == 7))
                for nb in range(2):
                    nc.tensor.matmul(
                        dps[:, 512 * nb:512 * (nb + 1)], ones_sb[:],
                        bd_sb[:, 512 * nb:512 * (nb + 1)],
                        start=False, stop=True)
                nc.vector.tensor_copy(drows[:], dps[:])

                for b in range(2):
                    fp = fp0 if b == 0 else fp1
                    ot = opool.tile([128, 1024], f32, tag="ot", name=f"ot{b}")
                    nc.vector.tensor_tensor(ot[:], fp[:], drows[:], Alu.add)
                    nc.scalar.dma_start(out_d[b], ot[:])

                for b in range(2, 8):
                    fp = psF.tile([128, 1024], f32, tag="fp", name=f"fp{b}")
                    for nb in range(2):
                        for kt in range(8):
                            nc.tensor.matmul(
                                fp[:, 512 * nb:512 * (nb + 1)],
                                aslice(b, kt), wslice(kt, nb),
                                start=(kt == 0), stop=(kt == 7))
                    ot = opool.tile([128, 1024], f32, tag="ot", name=f"ot{b}")
                    nc.vector.tensor_tensor(ot[:], fp[:], drows[:], Alu.add)
                    nc.scalar.dma_start(out_d[b], ot[:])

    nc.compile()
    return nc


def get_nc():
    if "nc" not in _CACHE:
        _CACHE["nc"] = _build()
    return _CACHE["nc"]



def run(inputs, trace=False, **kwargs):
    from concourse.bass_utils import run_bass_kernel_spmd
    nc = get_nc()
    in_maps = host_prep(inputs)
    res = run_bass_kernel_spmd(nc, in_maps, list(range(NC)), trace=trace, **kwargs)
    out = np.empty((B, N0, D0), np.float32)
    for k in range(NC):
        out[:, 128 * k:128 * (k + 1), :] = res.results[k]["out"]
    return out, res


def kernel(**inputs):
    out, _ = run(inputs)
    return out
